# revision 1
# baseline (speedup 1.0000x reference)
"""HQQ-quantized linear + LoRA (nn_HQQLinearLoRA) on 8 trn2 NeuronCores.

  out = x @ ((W_q - zero)*scale)^T + (x @ lora_A @ lora_B) * 2.0 + bias

Sharding: 4 token-groups (batch dim) x 2 out-feature-groups = 8 cores.
Each core computes out[b, :, og*2048:(og+1)*2048] for its (b, og).

Host passes pre-transposed views (layout-only transforms):
  - xT   [4096, 2048] f32 : x[b].T, rows permuted so each 128-row k-tile's
         partition p maps to quant-group g = p % 64 (uniform across tiles)
  - wqT  [4096, 2048] i32 : W_q[o-shard].T with the same row permutation
  - scaleT/zeroT [64, 2048] f32, loraA [4096,16] (permuted), loraB [16,2048],
    bias [1,2048]

Device: dequant W on DVE using a [128, 2048] scale/zero tile (valid for every
k-tile thanks to the permutation), cast x to fp16 in-flight via SWDGE DMA,
fp16 matmul with fp32 PSUM accumulation; LoRA + bias fold into the same PSUM
accumulation as one K=17 matmul.
"""

import sys

import numpy as np

sys.path.append("/opt/trn_rl_repo")

import concourse.bass as bass  # noqa: E402
import concourse.mybir as mybir  # noqa: E402
import concourse.tile as tile  # noqa: E402
from concourse import bacc  # noqa: E402
from concourse.bass_utils import run_bass_kernel_spmd  # noqa: E402

B, S, I, O, R = 4, 2048, 4096, 4096, 16
GS = 64
G = I // GS  # 64
NCORES = 8
OG = 2
O_SH = O // OG  # 2048
T = S  # 2048 tokens per core
KT = I // 128  # 32 k-tiles
TCH = 512  # token chunk
NTCH = T // TCH  # 4
OCH = 512  # o quarter (dequant-W granule)
NOCH = O_SH // OCH  # 4
SCALING = 2.0

F32 = mybir.dt.float32
F16 = mybir.dt.float16
I32 = mybir.dt.int32

TRACE = False
TRACE_KWARGS = {}
LAST_RESULTS = None


def _perm() -> np.ndarray:
    """Row order such that k-tile k, partition p holds input-feature
    i = (p % 64)*64 + 2k + p//64, i.e. quant group g(i) = p % 64."""
    p = np.arange(128)
    out = np.empty(I, dtype=np.int64)
    for k in range(KT):
        out[k * 128 + p] = (p % 64) * 64 + 2 * k + p // 64
    return out


PERM = _perm()

_nc_cache = None


def _build(no_mm=False, no_deq=False, no_x=False):
    nc = bacc.Bacc(None)
    xT_d = nc.dram_tensor("xT", [I, T], F32, kind="ExternalInput")
    wqT_d = nc.dram_tensor("wqT", [I, O_SH], I32, kind="ExternalInput")
    scaleT_d = nc.dram_tensor("scaleT", [G, O_SH], F32, kind="ExternalInput")
    zeroT_d = nc.dram_tensor("zeroT", [G, O_SH], F32, kind="ExternalInput")
    loraA_d = nc.dram_tensor("loraA", [I, R], F32, kind="ExternalInput")
    loraB_d = nc.dram_tensor("loraB", [R, O_SH], F32, kind="ExternalInput")
    bias_d = nc.dram_tensor("bias", [1, O_SH], F32, kind="ExternalInput")
    ones_d = nc.dram_tensor("ones", [1, T], F32, kind="ExternalInput")
    out_d = nc.dram_tensor("out", [NOCH, T, OCH], F32, kind="ExternalOutput")

    Copy = mybir.ActivationFunctionType.Copy

    with tile.TileContext(nc) as tc:
        with (
            tc.tile_pool(name="const", bufs=1) as constp,
            tc.tile_pool(name="w16", bufs=3) as w16p,
            tc.tile_pool(name="wq", bufs=3) as wqp,
            tc.tile_pool(name="d16", bufs=2) as d16p,
            tc.tile_pool(name="x16", bufs=2) as x16p,
            tc.tile_pool(name="ob", bufs=3) as obp,
            tc.tile_pool(name="ps", bufs=4, space="PSUM") as psp,
            tc.tile_pool(name="psl", bufs=2, space="PSUM") as pslp,
        ):
            # ---- constants ----
            # scale/zero expanded: row p <- scaleT[p % 64, :]
            s16 = constp.tile([128, O_SH], F16)
            z16 = constp.tile([128, O_SH], F16)
            for h in (0, 1):
                nc.gpsimd.dma_start(s16[64 * h : 64 * h + 64, :], scaleT_d[:, :])
                nc.gpsimd.dma_start(z16[64 * h : 64 * h + 64, :], zeroT_d[:, :])
            # lora_A tiles: [128, (k r)]; fold the 2.0 LoRA scaling here
            laf = constp.tile([128, KT, R], F32)
            nc.sync.dma_start(laf[:], loraA_d.rearrange("(k p) r -> p k r", p=128))
            la16 = constp.tile([128, KT, R], F16)
            nc.scalar.activation(la16[:], laf[:], Copy, scale=SCALING)
            # [loraB; bias] rhs for the fused K=17 matmul
            lb16 = constp.tile([R + 1, O_SH], F16)
            nc.gpsimd.dma_start(lb16[0:R, :], loraB_d[:])
            nc.gpsimd.dma_start(lb16[R : R + 1, :], bias_d[:])
            # [t1; ones] lhsT rows; row R stays 1.0
            t1sb = constp.tile([R + 1, T], F16)
            nc.gpsimd.dma_start(t1sb[R : R + 1, :], ones_d[:])

            for oh in range(2):  # o-halves (x streamed once per half)
                w16q_list = []
                for qq in range(2):  # dequant W in o-quarters of 512
                    oq = oh * 2 + qq
                    w16q = w16p.tile([128, KT, OCH], F16)
                    nc.gpsimd.dma_start(
                        w16q[:],
                        wqT_d[:, oq * OCH : (oq + 1) * OCH].rearrange(
                            "(k p) o -> p k o", p=128
                        ),
                    )
                    if not no_deq:
                        for k in range(KT):
                            d16 = d16p.tile([128, OCH], F16)
                            nc.vector.tensor_sub(
                                d16[:], w16q[:, k, :], z16[:, oq * OCH : (oq + 1) * OCH]
                            )
                            nc.vector.tensor_mul(
                                w16q[:, k, :],
                                d16[:],
                                s16[:, oq * OCH : (oq + 1) * OCH],
                            )
                    w16q_list.append(w16q)

                for tci in range(NTCH):
                    # one 8 MiB casting DMA per token chunk: f32 -> f16
                    x16 = x16p.tile([128, KT, TCH], F16)
                    if not no_x:
                        nc.gpsimd.dma_start(
                            x16[:],
                            xT_d[:, tci * TCH : (tci + 1) * TCH].rearrange(
                                "(k p) t -> p k t", p=128
                            ),
                        )
                    if oh == 0:
                        t1ps = pslp.tile([R, TCH], F32)
                        for k in range(KT):
                            nc.tensor.matmul(
                                t1ps[:],
                                la16[:, k, :],
                                x16[:, k, :],
                                start=(k == 0),
                                stop=(k == KT - 1),
                            )
                        nc.vector.tensor_copy(
                            t1sb[0:R, tci * TCH : (tci + 1) * TCH], t1ps[:]
                        )
                    for tt in range(TCH // 128):
                        t0 = tci * TCH + tt * 128
                        for qq in range(2):
                            o0 = (oh * 2 + qq) * OCH
                            ps = psp.tile([128, OCH], F32)
                            if not no_mm:
                                for k in range(KT):
                                    nc.tensor.matmul(
                                        ps[:],
                                        x16[:, k, tt * 128 : tt * 128 + 128],
                                        w16q_list[qq][:, k, :],
                                        start=(k == 0),
                                        stop=False,
                                    )
                            nc.tensor.matmul(
                                ps[:],
                                t1sb[:, t0 : t0 + 128],
                                lb16[:, o0 : o0 + OCH],
                                start=(no_mm),
                                stop=True,
                            )
                            ob = obp.tile([128, OCH], F32)
                            nc.vector.tensor_copy(ob[:], ps[:])
                            nc.sync.dma_start(out_d[oh * 2 + qq, t0 : t0 + 128, :], ob[:])

    nc.compile()
    return nc


def kernel(x, W_q, scale, zero, lora_A, lora_B, bias):
    global _nc_cache, LAST_RESULTS
    if _nc_cache is None:
        _nc_cache = _build()
    nc = _nc_cache

    x = np.asarray(x, dtype=np.float32)
    W_q = np.asarray(W_q, dtype=np.int32)
    scale = np.asarray(scale, dtype=np.float32)
    zero = np.asarray(zero, dtype=np.float32)
    lora_A = np.asarray(lora_A, dtype=np.float32)
    lora_B = np.asarray(lora_B, dtype=np.float32)
    bias = np.asarray(bias, dtype=np.float32)

    loraA_p = np.ascontiguousarray(lora_A[PERM])
    # xT per batch element (shared by the 2 o-group cores)
    xT_b = [np.ascontiguousarray(x[b].T[PERM]) for b in range(B)]

    in_maps = []
    for c in range(NCORES):
        b, og = c // OG, c % OG
        osl = slice(og * O_SH, (og + 1) * O_SH)
        in_maps.append(
            {
                "xT": xT_b[b],
                "wqT": np.ascontiguousarray(W_q[osl].T[PERM]),
                "scaleT": np.ascontiguousarray(scale[osl].T),
                "zeroT": np.ascontiguousarray(zero[osl].T),
                "loraA": loraA_p,
                "loraB": np.ascontiguousarray(lora_B[:, osl]),
                "bias": np.ascontiguousarray(bias[osl]).reshape(1, O_SH),
                "ones": np.ones((1, T), dtype=np.float32),
            }
        )

    res = run_bass_kernel_spmd(
        nc,
        in_maps,
        core_ids=list(range(NCORES)),
        trace=TRACE,
        trace_kwargs=TRACE_KWARGS,
    )
    LAST_RESULTS = res

    out = np.empty((B, S, O), dtype=np.float32)
    for c in range(NCORES):
        b, og = c // OG, c % OG
        o_c = res.results[c]["out"]  # [NOCH, T, OCH]
        for q in range(NOCH):
            o0 = og * O_SH + q * OCH
            out[b, :, o0 : o0 + OCH] = o_c[q]
    return out



# revision 2
# speedup vs baseline: 1.3127x; 1.3127x over previous
"""HQQ-quantized linear + LoRA (nn_HQQLinearLoRA) on 8 trn2 NeuronCores.

  out = x @ ((W_q - zero)*scale)^T + (x @ lora_A @ lora_B) * 2.0 + bias

Sharding: 4 token-groups (batch dim) x 2 out-feature-groups = 8 cores.
Each core computes out[b, :, og*2048:(og+1)*2048] for its (b, og).

Host passes pre-transposed views (layout-only transforms):
  - xT   [4096, 2048] f32 : x[b].T, rows permuted so each 128-row k-tile's
         partition p maps to quant-group g = p % 64 (uniform across tiles)
  - wqT  [4096, 2048] i32 : W_q[o-shard].T with the same row permutation
  - scaleT/zeroT [64, 2048] f32, loraA [4096,16] (permuted), loraB [16,2048],
    bias [1,2048]

Device: dequant W on DVE using a [128, 2048] scale/zero tile (valid for every
k-tile thanks to the permutation), cast x to fp16 in-flight via SWDGE DMA,
fp16 matmul with fp32 PSUM accumulation; LoRA + bias fold into the same PSUM
accumulation as one K=17 matmul.
"""

import sys

import numpy as np

sys.path.append("/opt/trn_rl_repo")

import concourse.bass as bass  # noqa: E402
import concourse.mybir as mybir  # noqa: E402
import concourse.tile as tile  # noqa: E402
from concourse import bacc  # noqa: E402
from concourse.bass_utils import run_bass_kernel_spmd  # noqa: E402

B, S, I, O, R = 4, 2048, 4096, 4096, 16
GS = 64
G = I // GS  # 64
NCORES = 8
OG = 2
O_SH = O // OG  # 2048
T = S  # 2048 tokens per core
KT = I // 128  # 32 k-tiles
TCH = 512  # token chunk
NTCH = T // TCH  # 4
OCH = 512  # o quarter (dequant-W granule)
NOCH = O_SH // OCH  # 4
SCALING = 2.0

F32 = mybir.dt.float32
F16 = mybir.dt.bfloat16
I32 = mybir.dt.int32

TRACE = False
TRACE_KWARGS = {}
LAST_RESULTS = None


def _perm() -> np.ndarray:
    """Row order such that k-tile k, partition p holds input-feature
    i = (p % 64)*64 + 2k + p//64, i.e. quant group g(i) = p % 64."""
    p = np.arange(128)
    out = np.empty(I, dtype=np.int64)
    for k in range(KT):
        out[k * 128 + p] = (p % 64) * 64 + 2 * k + p // 64
    return out


PERM = _perm()

_nc_cache = None


def _build(no_mm=False, no_deq=False, no_x=False):
    nc = bacc.Bacc(None)
    xT_d = nc.dram_tensor("xT", [I, T], F32, kind="ExternalInput")
    wqT_d = nc.dram_tensor("wqT", [I, O_SH], I32, kind="ExternalInput")
    scaleT_d = nc.dram_tensor("scaleT", [G, O_SH], F32, kind="ExternalInput")
    zeroT_d = nc.dram_tensor("zeroT", [G, O_SH], F32, kind="ExternalInput")
    loraA_d = nc.dram_tensor("loraA", [I, R], F32, kind="ExternalInput")
    loraB_d = nc.dram_tensor("loraB", [R, O_SH], F32, kind="ExternalInput")
    bias_d = nc.dram_tensor("bias", [1, O_SH], F32, kind="ExternalInput")
    ones_d = nc.dram_tensor("ones", [1, T], F32, kind="ExternalInput")
    out_d = nc.dram_tensor("out", [NOCH, T, OCH], F32, kind="ExternalOutput")

    Copy = mybir.ActivationFunctionType.Copy

    with tile.TileContext(nc) as tc:
        with (
            tc.tile_pool(name="const", bufs=1) as constp,
            tc.tile_pool(name="w16", bufs=3) as w16p,
            tc.tile_pool(name="wq", bufs=3) as wqp,
            tc.tile_pool(name="d16", bufs=2) as d16p,
            tc.tile_pool(name="x16", bufs=2) as x16p,
            tc.tile_pool(name="ob", bufs=3) as obp,
            tc.tile_pool(name="ps", bufs=4, space="PSUM") as psp,
            tc.tile_pool(name="psl", bufs=2, space="PSUM") as pslp,
        ):
            # ---- constants ----
            # scale/zero expanded: row p <- scaleT[p % 64, :]
            s16 = constp.tile([128, O_SH], F16)
            z16 = constp.tile([128, O_SH], F16)
            for h in (0, 1):
                nc.gpsimd.dma_start(s16[64 * h : 64 * h + 64, :], scaleT_d[:, :])
                nc.gpsimd.dma_start(z16[64 * h : 64 * h + 64, :], zeroT_d[:, :])
            # lora_A tiles: [128, (k r)]; fold the 2.0 LoRA scaling here
            laf = constp.tile([128, KT, R], F32)
            nc.sync.dma_start(laf[:], loraA_d.rearrange("(k p) r -> p k r", p=128))
            la16 = constp.tile([128, KT, R], F16)
            nc.scalar.activation(la16[:], laf[:], Copy, scale=SCALING)
            # [loraB; bias] rhs for the fused K=17 matmul
            lb16 = constp.tile([R + 1, O_SH], F16)
            nc.gpsimd.dma_start(lb16[0:R, :], loraB_d[:])
            nc.gpsimd.dma_start(lb16[R : R + 1, :], bias_d[:])
            # [t1; ones] lhsT rows; row R stays 1.0
            t1sb = constp.tile([R + 1, T], F16)
            nc.gpsimd.dma_start(t1sb[R : R + 1, :], ones_d[:])

            for oh in range(2):  # o-halves (x streamed once per half)
                w16q_list = []
                for qq in range(2):  # dequant W in o-quarters of 512
                    oq = oh * 2 + qq
                    w16q = w16p.tile([128, KT, OCH], F16)
                    nc.gpsimd.dma_start(
                        w16q[:],
                        wqT_d[:, oq * OCH : (oq + 1) * OCH].rearrange(
                            "(k p) o -> p k o", p=128
                        ),
                    )
                    if not no_deq:
                        for k in range(KT):
                            d16 = d16p.tile([128, OCH], F16)
                            nc.vector.tensor_sub(
                                d16[:], w16q[:, k, :], z16[:, oq * OCH : (oq + 1) * OCH]
                            )
                            nc.vector.tensor_mul(
                                w16q[:, k, :],
                                d16[:],
                                s16[:, oq * OCH : (oq + 1) * OCH],
                            )
                    w16q_list.append(w16q)

                for tci in range(NTCH):
                    # one 8 MiB casting DMA per token chunk: f32 -> f16
                    x16 = x16p.tile([128, KT, TCH], F16)
                    if not no_x:
                        nc.gpsimd.dma_start(
                            x16[:],
                            xT_d[:, tci * TCH : (tci + 1) * TCH].rearrange(
                                "(k p) t -> p k t", p=128
                            ),
                        )
                    if oh == 0:
                        t1ps = pslp.tile([R, TCH], F32)
                        for k in range(KT):
                            nc.tensor.matmul(
                                t1ps[:],
                                la16[:, k, :],
                                x16[:, k, :],
                                start=(k == 0),
                                stop=(k == KT - 1),
                            )
                        nc.vector.tensor_copy(
                            t1sb[0:R, tci * TCH : (tci + 1) * TCH], t1ps[:]
                        )
                    for tt in range(TCH // 128):
                        t0 = tci * TCH + tt * 128
                        for qq in range(2):
                            o0 = (oh * 2 + qq) * OCH
                            ps = psp.tile([128, OCH], F32)
                            if not no_mm:
                                for k in range(KT):
                                    nc.tensor.matmul(
                                        ps[:],
                                        x16[:, k, tt * 128 : tt * 128 + 128],
                                        w16q_list[qq][:, k, :],
                                        start=(k == 0),
                                        stop=False,
                                    )
                            nc.tensor.matmul(
                                ps[:],
                                t1sb[:, t0 : t0 + 128],
                                lb16[:, o0 : o0 + OCH],
                                start=(no_mm),
                                stop=True,
                            )
                            ob = obp.tile([128, OCH], F32)
                            nc.vector.tensor_copy(ob[:], ps[:])
                            nc.sync.dma_start(out_d[oh * 2 + qq, t0 : t0 + 128, :], ob[:])

    nc.compile()
    return nc


def kernel(x, W_q, scale, zero, lora_A, lora_B, bias):
    global _nc_cache, LAST_RESULTS
    if _nc_cache is None:
        _nc_cache = _build()
    nc = _nc_cache

    x = np.asarray(x, dtype=np.float32)
    W_q = np.asarray(W_q, dtype=np.int32)
    scale = np.asarray(scale, dtype=np.float32)
    zero = np.asarray(zero, dtype=np.float32)
    lora_A = np.asarray(lora_A, dtype=np.float32)
    lora_B = np.asarray(lora_B, dtype=np.float32)
    bias = np.asarray(bias, dtype=np.float32)

    loraA_p = np.ascontiguousarray(lora_A[PERM])
    # xT per batch element (shared by the 2 o-group cores)
    xT_b = [np.ascontiguousarray(x[b].T[PERM]) for b in range(B)]

    in_maps = []
    for c in range(NCORES):
        b, og = c // OG, c % OG
        osl = slice(og * O_SH, (og + 1) * O_SH)
        in_maps.append(
            {
                "xT": xT_b[b],
                "wqT": np.ascontiguousarray(W_q[osl].T[PERM]),
                "scaleT": np.ascontiguousarray(scale[osl].T),
                "zeroT": np.ascontiguousarray(zero[osl].T),
                "loraA": loraA_p,
                "loraB": np.ascontiguousarray(lora_B[:, osl]),
                "bias": np.ascontiguousarray(bias[osl]).reshape(1, O_SH),
                "ones": np.ones((1, T), dtype=np.float32),
            }
        )

    res = run_bass_kernel_spmd(
        nc,
        in_maps,
        core_ids=list(range(NCORES)),
        trace=TRACE,
        trace_kwargs=TRACE_KWARGS,
    )
    LAST_RESULTS = res

    out = np.empty((B, S, O), dtype=np.float32)
    for c in range(NCORES):
        b, og = c // OG, c % OG
        o_c = res.results[c]["out"]  # [NOCH, T, OCH]
        for q in range(NOCH):
            o0 = og * O_SH + q * OCH
            out[b, :, o0 : o0 + OCH] = o_c[q]
    return out



# revision 3
# speedup vs baseline: 1.3179x; 1.0040x over previous
"""HQQ-quantized linear + LoRA (nn_HQQLinearLoRA) on 8 trn2 NeuronCores.

  out = x @ ((W_q - zero)*scale)^T + (x @ lora_A @ lora_B) * 2.0 + bias

Sharding: 4 token-groups (batch dim) x 2 out-feature-groups = 8 cores.
Each core computes out[b, :, og*2048:(og+1)*2048] for its (b, og).

Host passes layout/dtype-transformed views only:
  - xh   [128, 8, 32, 256] bf16 : x[b].T row-permuted (k-tile partition p
         maps to quant group g = p % 64) and chunk-blocked so each token
         chunk is one fully contiguous per-partition DMA
  - wq   [128, 4, 32, 512] bf16 : W_q[o-shard].T same permutation, o-quarter
         blocked (values 0..15 are exact in bf16 - pure dtype transform)
  - sb/zb [128, 2048] bf16      : scale/zero rows pre-broadcast to p%64
  - la   [128, 32, 16] bf16     : (2*lora_A) permuted, k-tile blocked
  - lb   [17, 2048] bf16        : [lora_B; bias]
  - sel  [128, 16] bf16         : strip-sum selector (sel[32j+r, r] = 1)

Device: W quarters DMA'd on the Scalar HWDGE ring (x/out own the Sync ring),
dequant (w-z)*s in place on DVE; x chunks stream on Sync; fp32-PSUM
accumulation over 32 k-tiles plus a fused K=17 matmul adding LoRA+bias;
PSUM evacuated on the Scalar engine. t1 = x@(2 lora_A) runs 4 k-tiles
concurrently via tile_position column strips, strip sums combined by a
selector matmul. The second half's W load/dequant is emitted mid-loop so
it overlaps first-half compute without joining the startup DMA burst.
"""

import sys

import numpy as np

sys.path.append("/opt/trn_rl_repo")

import ml_dtypes  # noqa: E402

import concourse.bass as bass  # noqa: E402
import concourse.mybir as mybir  # noqa: E402
import concourse.tile as tile  # noqa: E402
from concourse import bacc  # noqa: E402
from concourse.bass_utils import run_bass_kernel_spmd  # noqa: E402

B, S, I, O, R = 4, 2048, 4096, 4096, 16
GS = 64
NCORES = 8
OG = 2
O_SH = O // OG  # 2048
T = S  # 2048 tokens per core
KT = I // 128  # 32 k-tiles
TCH = 256  # token chunk
NTCH = T // TCH  # 8
NQ = 4  # o-quarters of 512
SCALING = 2.0

F32 = mybir.dt.float32
DT16 = mybir.dt.bfloat16
NP16 = ml_dtypes.bfloat16

TRACE = False
TRACE_KWARGS = {}
LAST_RESULTS = None


def _perm() -> np.ndarray:
    """Row order such that k-tile k, partition p holds input-feature
    i = (p % 64)*64 + 2k + p//64, i.e. quant group g(i) = p % 64."""
    p = np.arange(128)
    out = np.empty(I, dtype=np.int64)
    for k in range(KT):
        out[k * 128 + p] = (p % 64) * 64 + 2 * k + p // 64
    return out


PERM = _perm()

_nc_cache = None


def _build():
    nc = bacc.Bacc(None)
    xh_d = nc.dram_tensor("xh", [128, NTCH, KT, TCH], DT16, kind="ExternalInput")
    wq_d = nc.dram_tensor("wq", [128, NQ, KT, 512], DT16, kind="ExternalInput")
    sb_d = nc.dram_tensor("sb", [128, O_SH], DT16, kind="ExternalInput")
    zb_d = nc.dram_tensor("zb", [128, O_SH], DT16, kind="ExternalInput")
    la_d = nc.dram_tensor("la", [128, KT, R], DT16, kind="ExternalInput")
    lb_d = nc.dram_tensor("lb", [R + 1, O_SH], DT16, kind="ExternalInput")
    ones_d = nc.dram_tensor("ones", [1, T], DT16, kind="ExternalInput")
    sel_d = nc.dram_tensor("sel", [128, R], DT16, kind="ExternalInput")
    out_d = nc.dram_tensor("out", [NQ, T, 512], F32, kind="ExternalOutput")

    with tile.TileContext(nc) as tc:
        with (
            tc.tile_pool(name="const", bufs=1) as constp,
            tc.tile_pool(name="w16", bufs=4) as w16p,
            tc.tile_pool(name="d16", bufs=3) as d16p,
            tc.tile_pool(name="x16", bufs=2) as x16p,
            tc.tile_pool(name="t1c", bufs=2) as t1cp,
            tc.tile_pool(name="ob", bufs=3) as obp,
            tc.tile_pool(name="ps", bufs=4, space="PSUM") as psp,
            tc.tile_pool(name="psl", bufs=2, space="PSUM") as pslp,
        ):
            # sync-ring order: x chunk 0 first, then the small constants
            x0 = x16p.tile([128, KT, TCH], DT16, name="x16")
            nc.sync.dma_start(x0[:], xh_d[:, 0])
            la16 = constp.tile([128, KT, R], DT16)
            nc.sync.dma_start(la16[:], la_d[:])
            sel16 = constp.tile([128, R], DT16)
            nc.sync.dma_start(sel16[:], sel_d[:])
            t1sb = constp.tile([R + 1, T], DT16)
            nc.sync.dma_start(t1sb[R : R + 1, :], ones_d[:])
            lb16 = constp.tile([R + 1, O_SH], DT16)
            nc.sync.dma_start(lb16[:], lb_d[:])
            s16 = constp.tile([128, O_SH], DT16)
            z16 = constp.tile([128, O_SH], DT16)
            nc.sync.dma_start(s16[:], sb_d[:])
            nc.sync.dma_start(z16[:], zb_d[:])

            def t1_chain(x16, tci):
                # 4 k-tiles concurrently in 32-wide column strips
                t1p = pslp.tile([128, TCH], F32, tag="t1")
                for kk in range(KT // 4):
                    for j in range(4):
                        nc.tensor.matmul(
                            t1p[32 * j : 32 * j + R, :],
                            la16[:, kk * 4 + j, :],
                            x16[:, kk * 4 + j, :],
                            start=(kk == 0),
                            stop=(kk == KT // 4 - 1),
                            tile_position=(0, 32 * j),
                        )
                t1c = t1cp.tile([128, TCH], DT16)
                nc.vector.tensor_copy(t1c[:], t1p[:])
                t1f = pslp.tile([R, TCH], F32, tag="t1")
                nc.tensor.matmul(t1f[:], sel16[:], t1c[:], start=True, stop=True)
                nc.vector.tensor_copy(t1sb[0:R, tci * TCH : (tci + 1) * TCH], t1f[:])

            def load_quarter(q):
                w16 = w16p.tile([128, KT, 512], DT16)
                nc.scalar.dma_start(w16[:], wq_d[:, q])
                for k in range(KT):
                    d16 = d16p.tile([128, 512], DT16)
                    nc.vector.tensor_sub(
                        d16[:], w16[:, k, :], z16[:, q * 512 : (q + 1) * 512]
                    )
                    nc.vector.tensor_mul(
                        w16[:, k, :], d16[:], s16[:, q * 512 : (q + 1) * 512]
                    )
                return w16

            t1_chain(x0, 0)

            wt = [load_quarter(0), load_quarter(1), None, None]

            for oh in range(2):
                for tci in range(NTCH):
                    if oh == 0 and tci == 0:
                        x16 = x0
                    else:
                        x16 = x16p.tile([128, KT, TCH], DT16)
                        nc.sync.dma_start(x16[:], xh_d[:, tci])
                        if oh == 0:
                            t1_chain(x16, tci)
                    if oh == 0 and tci in (2, 3):
                        wt[tci] = load_quarter(tci)
                    for tt in range(TCH // 128):
                        t0 = tci * TCH + tt * 128
                        for qq in range(2):
                            q = oh * 2 + qq
                            ps = psp.tile([128, 512], F32)
                            for k in range(KT):
                                nc.tensor.matmul(
                                    ps[:],
                                    x16[:, k, tt * 128 : tt * 128 + 128],
                                    wt[q][:, k, :],
                                    start=(k == 0),
                                    stop=False,
                                )
                            nc.tensor.matmul(
                                ps[:],
                                t1sb[:, t0 : t0 + 128],
                                lb16[:, q * 512 : (q + 1) * 512],
                                start=False,
                                stop=True,
                            )
                            ob = obp.tile([128, 512], F32)
                            nc.scalar.copy(ob[:], ps[:])
                            nc.sync.dma_start(out_d[q, t0 : t0 + 128, :], ob[:])

    nc.compile()
    return nc


def kernel(x, W_q, scale, zero, lora_A, lora_B, bias):
    global _nc_cache, LAST_RESULTS
    if _nc_cache is None:
        _nc_cache = _build()
    nc = _nc_cache

    x = np.asarray(x, dtype=np.float32)
    W_q = np.asarray(W_q, dtype=np.int32)
    scale = np.asarray(scale, dtype=np.float32)
    zero = np.asarray(zero, dtype=np.float32)
    lora_A = np.asarray(lora_A, dtype=np.float32)
    lora_B = np.asarray(lora_B, dtype=np.float32)
    bias = np.asarray(bias, dtype=np.float32)

    xh_b = []
    for b in range(B):
        xT = x[b].T[PERM]  # [I, T]
        xh = xT.reshape(KT, 128, NTCH, TCH).transpose(1, 2, 0, 3)
        xh_b.append(np.ascontiguousarray(xh.astype(NP16)))
    la = (SCALING * lora_A)[PERM].reshape(KT, 128, R).transpose(1, 0, 2)
    la = np.ascontiguousarray(la.astype(NP16))
    sel = np.zeros((128, R), dtype=NP16)
    for j in range(4):
        sel[32 * j + np.arange(R), np.arange(R)] = 1

    in_maps = []
    for c in range(NCORES):
        b, og = c // OG, c % OG
        osl = slice(og * O_SH, (og + 1) * O_SH)
        wq = W_q[osl].T[PERM].reshape(KT, 128, NQ, 512).transpose(1, 2, 0, 3)
        sb = np.ascontiguousarray(scale[osl].T)[np.arange(128) % GS]
        zb = np.ascontiguousarray(zero[osl].T)[np.arange(128) % GS]
        lb = np.concatenate([lora_B[:, osl], bias[None, osl]], axis=0)
        in_maps.append(
            {
                "xh": xh_b[b],
                "wq": np.ascontiguousarray(wq.astype(NP16)),
                "sb": np.ascontiguousarray(sb.astype(NP16)),
                "zb": np.ascontiguousarray(zb.astype(NP16)),
                "la": la,
                "lb": np.ascontiguousarray(lb.astype(NP16)),
                "ones": np.ones((1, T), dtype=NP16),
                "sel": sel,
            }
        )

    res = run_bass_kernel_spmd(
        nc,
        in_maps,
        core_ids=list(range(NCORES)),
        trace=TRACE,
        trace_kwargs=TRACE_KWARGS,
    )
    LAST_RESULTS = res

    out = np.empty((B, S, O), dtype=np.float32)
    for c in range(NCORES):
        b, og = c // OG, c % OG
        o_c = res.results[c]["out"]  # [NQ, T, 512]
        for q in range(NQ):
            o0 = og * O_SH + q * 512
            out[b, :, o0 : o0 + 512] = o_c[q]
    return out


# revision 4
# speedup vs baseline: 1.3230x; 1.0038x over previous
"""HQQ-quantized linear + LoRA (nn_HQQLinearLoRA) on 8 trn2 NeuronCores.

  out = x @ ((W_q - zero)*scale)^T + (x @ lora_A @ lora_B) * 2.0 + bias

Sharding: 4 token-groups (batch dim) x 2 out-feature-groups = 8 cores.
Each core computes out[b, :, og*2048:(og+1)*2048] for its (b, og).

Host passes layout/dtype-transformed views only:
  - xh   [128, 8, 32, 256] bf16 : x[b].T row-permuted (k-tile partition p
         maps to quant group g = p % 64) and chunk-blocked so each token
         chunk is one fully contiguous per-partition DMA
  - wq   [128, 4, 32, 512] bf16 : W_q[o-shard].T same permutation, o-quarter
         blocked (values 0..15 are exact in bf16 - pure dtype transform)
  - sb/zb [128, 2048] bf16      : scale/zero rows pre-broadcast to p%64
  - la   [128, 32, 16] bf16     : (2*lora_A) permuted, k-tile blocked
  - lb   [17, 2048] bf16        : [lora_B; bias]
  - sel  [128, 16] bf16         : strip-sum selector (sel[32j+r, r] = 1)

Device: W quarters DMA'd on the Scalar HWDGE ring (x/out own the Sync ring),
dequant (w-z)*s in place on DVE; x chunks stream on Sync; fp32-PSUM
accumulation over 32 k-tiles plus a fused K=17 matmul adding LoRA+bias;
PSUM evacuated on the Scalar engine. t1 = x@(2 lora_A) runs 4 k-tiles
concurrently via tile_position column strips, strip sums combined by a
selector matmul. The second half's W load/dequant is emitted mid-loop so
it overlaps first-half compute without joining the startup DMA burst.
"""

import sys

import numpy as np

sys.path.append("/opt/trn_rl_repo")

import ml_dtypes  # noqa: E402

import concourse.bass as bass  # noqa: E402
import concourse.mybir as mybir  # noqa: E402
import concourse.tile as tile  # noqa: E402
from concourse import bacc  # noqa: E402
from concourse.bass_utils import run_bass_kernel_spmd  # noqa: E402

B, S, I, O, R = 4, 2048, 4096, 4096, 16
GS = 64
NCORES = 8
OG = 2
O_SH = O // OG  # 2048
T = S  # 2048 tokens per core
KT = I // 128  # 32 k-tiles
TCH = 256  # token chunk
NTCH = T // TCH  # 8
NQ = 4  # o-quarters of 512
SCALING = 2.0

F32 = mybir.dt.float32
DT16 = mybir.dt.bfloat16
NP16 = ml_dtypes.bfloat16

TRACE = False
TRACE_KWARGS = {}
LAST_RESULTS = None


def _perm() -> np.ndarray:
    """Row order such that k-tile k, partition p holds input-feature
    i = (p % 64)*64 + 2k + p//64, i.e. quant group g(i) = p % 64."""
    p = np.arange(128)
    out = np.empty(I, dtype=np.int64)
    for k in range(KT):
        out[k * 128 + p] = (p % 64) * 64 + 2 * k + p // 64
    return out


PERM = _perm()

_nc_cache = None


def _build():
    nc = bacc.Bacc(None)
    xh_d = nc.dram_tensor("xh", [128, NTCH, KT, TCH], DT16, kind="ExternalInput")
    wq_d = nc.dram_tensor("wq", [128, NQ, KT, 512], DT16, kind="ExternalInput")
    sb_d = nc.dram_tensor("sb", [128, O_SH], DT16, kind="ExternalInput")
    zb_d = nc.dram_tensor("zb", [128, O_SH], DT16, kind="ExternalInput")
    la_d = nc.dram_tensor("la", [128, KT, R], DT16, kind="ExternalInput")
    lb_d = nc.dram_tensor("lb", [R + 1, O_SH], DT16, kind="ExternalInput")
    ones_d = nc.dram_tensor("ones", [1, T], DT16, kind="ExternalInput")
    sel_d = nc.dram_tensor("sel", [128, R], DT16, kind="ExternalInput")
    out_d = nc.dram_tensor("out", [NQ, T, 512], F32, kind="ExternalOutput")

    with tile.TileContext(nc) as tc:
        with (
            tc.tile_pool(name="const", bufs=1) as constp,
            tc.tile_pool(name="w16", bufs=4) as w16p,
            tc.tile_pool(name="d16", bufs=3) as d16p,
            tc.tile_pool(name="x16", bufs=2) as x16p,
            tc.tile_pool(name="t1c", bufs=2) as t1cp,
            tc.tile_pool(name="ob", bufs=3) as obp,
            tc.tile_pool(name="ps", bufs=4, space="PSUM") as psp,
            tc.tile_pool(name="psl", bufs=2, space="PSUM") as pslp,
        ):
            # sync-ring order: x chunk 0 first, then the small constants
            x0 = x16p.tile([128, KT, TCH], DT16, name="x16")
            nc.sync.dma_start(x0[:], xh_d[:, 0])
            la16 = constp.tile([128, KT, R], DT16)
            nc.sync.dma_start(la16[:], la_d[:])
            sel16 = constp.tile([128, R], DT16)
            nc.sync.dma_start(sel16[:], sel_d[:])
            # t1sb/lb padded to full 128 partitions (zero rows contribute 0)
            # so the fused LoRA+bias matmul is a uniform K=128 FWL-eligible op
            t1sb = constp.tile([128, T], DT16)
            nc.vector.memset(t1sb[:], 0.0)
            nc.sync.dma_start(t1sb[R : R + 1, :], ones_d[:])
            lb16 = constp.tile([128, O_SH], DT16)
            nc.vector.memset(lb16[:], 0.0)
            nc.sync.dma_start(lb16[0 : R + 1, :], lb_d[:])
            s16 = constp.tile([128, O_SH], DT16)
            z16 = constp.tile([128, O_SH], DT16)
            nc.sync.dma_start(s16[:], sb_d[:])
            nc.sync.dma_start(z16[:], zb_d[:])

            def t1_chain(x16, tci):
                # 4 k-tiles concurrently in 32-wide column strips
                t1p = pslp.tile([128, TCH], F32, tag="t1")
                for kk in range(KT // 4):
                    for j in range(4):
                        nc.tensor.matmul(
                            t1p[32 * j : 32 * j + R, :],
                            la16[:, kk * 4 + j, :],
                            x16[:, kk * 4 + j, :],
                            start=(kk == 0),
                            stop=(kk == KT // 4 - 1),
                            tile_position=(0, 32 * j),
                        )
                t1c = t1cp.tile([128, TCH], DT16)
                nc.vector.tensor_copy(t1c[:], t1p[:])
                t1f = pslp.tile([R, TCH], F32, tag="t1")
                nc.tensor.matmul(t1f[:], sel16[:], t1c[:], start=True, stop=True)
                nc.vector.tensor_copy(t1sb[0:R, tci * TCH : (tci + 1) * TCH], t1f[:])

            def load_quarter(q):
                w16 = w16p.tile([128, KT, 512], DT16)
                nc.scalar.dma_start(w16[:], wq_d[:, q])
                for k in range(KT):
                    d16 = d16p.tile([128, 512], DT16)
                    nc.vector.tensor_sub(
                        d16[:], w16[:, k, :], z16[:, q * 512 : (q + 1) * 512]
                    )
                    nc.vector.tensor_mul(
                        w16[:, k, :], d16[:], s16[:, q * 512 : (q + 1) * 512]
                    )
                return w16

            t1_chain(x0, 0)

            wt = [load_quarter(0), load_quarter(1), None, None]

            for oh in range(2):
                for tci in range(NTCH):
                    if oh == 0 and tci == 0:
                        x16 = x0
                    else:
                        x16 = x16p.tile([128, KT, TCH], DT16)
                        nc.sync.dma_start(x16[:], xh_d[:, tci])
                        if oh == 0:
                            t1_chain(x16, tci)
                    if oh == 0 and tci in (2, 3):
                        wt[tci] = load_quarter(tci)
                    for tt in range(TCH // 128):
                        t0 = tci * TCH + tt * 128
                        for qq in range(2):
                            q = oh * 2 + qq
                            ps = psp.tile([128, 512], F32)
                            for k in range(KT):
                                nc.tensor.matmul(
                                    ps[:],
                                    x16[:, k, tt * 128 : tt * 128 + 128],
                                    wt[q][:, k, :],
                                    start=(k == 0),
                                    stop=False,
                                )
                            nc.tensor.matmul(
                                ps[:],
                                t1sb[:, t0 : t0 + 128],
                                lb16[:, q * 512 : (q + 1) * 512],
                                start=False,
                                stop=True,
                            )
                            ob = obp.tile([128, 512], F32)
                            nc.scalar.copy(ob[:], ps[:])
                            nc.sync.dma_start(out_d[q, t0 : t0 + 128, :], ob[:])

    nc.compile()
    return nc


def kernel(x, W_q, scale, zero, lora_A, lora_B, bias):
    global _nc_cache, LAST_RESULTS
    if _nc_cache is None:
        _nc_cache = _build()
    nc = _nc_cache

    x = np.asarray(x, dtype=np.float32)
    W_q = np.asarray(W_q, dtype=np.int32)
    scale = np.asarray(scale, dtype=np.float32)
    zero = np.asarray(zero, dtype=np.float32)
    lora_A = np.asarray(lora_A, dtype=np.float32)
    lora_B = np.asarray(lora_B, dtype=np.float32)
    bias = np.asarray(bias, dtype=np.float32)

    xh_b = []
    for b in range(B):
        xT = x[b].T[PERM]  # [I, T]
        xh = xT.reshape(KT, 128, NTCH, TCH).transpose(1, 2, 0, 3)
        xh_b.append(np.ascontiguousarray(xh.astype(NP16)))
    la = (SCALING * lora_A)[PERM].reshape(KT, 128, R).transpose(1, 0, 2)
    la = np.ascontiguousarray(la.astype(NP16))
    sel = np.zeros((128, R), dtype=NP16)
    for j in range(4):
        sel[32 * j + np.arange(R), np.arange(R)] = 1

    in_maps = []
    for c in range(NCORES):
        b, og = c // OG, c % OG
        osl = slice(og * O_SH, (og + 1) * O_SH)
        wq = W_q[osl].T[PERM].reshape(KT, 128, NQ, 512).transpose(1, 2, 0, 3)
        sb = np.ascontiguousarray(scale[osl].T)[np.arange(128) % GS]
        zb = np.ascontiguousarray(zero[osl].T)[np.arange(128) % GS]
        lb = np.concatenate([lora_B[:, osl], bias[None, osl]], axis=0)
        in_maps.append(
            {
                "xh": xh_b[b],
                "wq": np.ascontiguousarray(wq.astype(NP16)),
                "sb": np.ascontiguousarray(sb.astype(NP16)),
                "zb": np.ascontiguousarray(zb.astype(NP16)),
                "la": la,
                "lb": np.ascontiguousarray(lb.astype(NP16)),
                "ones": np.ones((1, T), dtype=NP16),
                "sel": sel,
            }
        )

    res = run_bass_kernel_spmd(
        nc,
        in_maps,
        core_ids=list(range(NCORES)),
        trace=TRACE,
        trace_kwargs=TRACE_KWARGS,
    )
    LAST_RESULTS = res

    out = np.empty((B, S, O), dtype=np.float32)
    for c in range(NCORES):
        b, og = c // OG, c % OG
        o_c = res.results[c]["out"]  # [NQ, T, 512]
        for q in range(NQ):
            o0 = og * O_SH + q * 512
            out[b, :, o0 : o0 + 512] = o_c[q]
    return out


# revision 5
# speedup vs baseline: 1.3624x; 1.0298x over previous
"""HQQ-quantized linear + LoRA (nn_HQQLinearLoRA) on 8 trn2 NeuronCores.

  out = x @ ((W_q - zero)*scale)^T + (x @ lora_A @ lora_B) * 2.0 + bias

Sharding: 4 token-groups (batch dim) x 2 out-feature-groups = 8 cores.
Each core computes out[b, :, og*2048:(og+1)*2048] for its (b, og).

Host passes layout/dtype-transformed views only:
  - xh   [128, 8, 32, 256] bf16 : x[b].T row-permuted (k-tile partition p
         maps to quant group g = p % 64) and chunk-blocked so each token
         chunk is one fully contiguous per-partition DMA
  - wq   [128, 4, 32, 512] bf16 : W_q[o-shard].T same permutation, o-quarter
         blocked (values 0..15 are exact in bf16 - pure dtype transform)
  - sb/zb [128, 2048] bf16      : scale/zero rows pre-broadcast to p%64
  - la   [128, 32, 16] bf16     : (2*lora_A) permuted, k-tile blocked
  - lb   [17, 2048] bf16        : [lora_B; bias]
  - sel  [128, 16] bf16         : strip-sum selector (sel[32j+r, r] = 1)

Device: W quarters DMA'd on the Scalar HWDGE ring (x/out own the Sync ring),
dequant (w-z)*s in place on DVE; x chunks stream on Sync; fp32-PSUM
accumulation over 32 k-tiles plus a fused K=17 matmul adding LoRA+bias;
PSUM evacuated on the Scalar engine. t1 = x@(2 lora_A) runs 4 k-tiles
concurrently via tile_position column strips, strip sums combined by a
selector matmul. The second half's W load/dequant is emitted mid-loop so
it overlaps first-half compute without joining the startup DMA burst.
"""

import sys

import numpy as np

sys.path.append("/opt/trn_rl_repo")

import ml_dtypes  # noqa: E402

import concourse.bass as bass  # noqa: E402
import concourse.mybir as mybir  # noqa: E402
import concourse.tile as tile  # noqa: E402
from concourse import bacc  # noqa: E402
from concourse.bass_utils import run_bass_kernel_spmd  # noqa: E402

B, S, I, O, R = 4, 2048, 4096, 4096, 16
GS = 64
NCORES = 8
OG = 2
O_SH = O // OG  # 2048
T = S  # 2048 tokens per core
KT = I // 128  # 32 k-tiles
TCH = 256  # token chunk
NTCH = T // TCH  # 8
NQ = 4  # o-quarters of 512
SCALING = 2.0

F32 = mybir.dt.float32
DT16 = mybir.dt.bfloat16
NP16 = ml_dtypes.bfloat16

TRACE = False
TRACE_KWARGS = {}
LAST_RESULTS = None


def _perm() -> np.ndarray:
    """Row order such that k-tile k, partition p holds input-feature
    i = (p % 64)*64 + 2k + p//64, i.e. quant group g(i) = p % 64."""
    p = np.arange(128)
    out = np.empty(I, dtype=np.int64)
    for k in range(KT):
        out[k * 128 + p] = (p % 64) * 64 + 2 * k + p // 64
    return out


PERM = _perm()

_nc_cache = None


def _build():
    nc = bacc.Bacc(None)
    xh_d = nc.dram_tensor("xh", [128, NTCH, KT, TCH], DT16, kind="ExternalInput")
    wq_d = nc.dram_tensor("wq", [128, NQ, KT, 512], DT16, kind="ExternalInput")
    sb_d = nc.dram_tensor("sb", [128, O_SH], DT16, kind="ExternalInput")
    zb_d = nc.dram_tensor("zb", [128, O_SH], DT16, kind="ExternalInput")
    la_d = nc.dram_tensor("la", [128, KT, R], DT16, kind="ExternalInput")
    lb_d = nc.dram_tensor("lb", [R + 1, O_SH], DT16, kind="ExternalInput")
    ones_d = nc.dram_tensor("ones", [1, T], DT16, kind="ExternalInput")
    sel_d = nc.dram_tensor("sel", [128, R], DT16, kind="ExternalInput")
    out_d = nc.dram_tensor("out", [NQ, T, 512], F32, kind="ExternalOutput")

    with tile.TileContext(nc) as tc:
        with (
            tc.tile_pool(name="const", bufs=1) as constp,
            tc.tile_pool(name="w16", bufs=4) as w16p,
            tc.tile_pool(name="x16", bufs=2) as x16p,
            tc.tile_pool(name="t1c", bufs=2) as t1cp,
            tc.tile_pool(name="ob", bufs=3) as obp,
            tc.tile_pool(name="ps", bufs=4, space="PSUM") as psp,
            tc.tile_pool(name="psl", bufs=2, space="PSUM") as pslp,
        ):
            # sync-ring order: x chunk 0 first, then the small constants
            x0 = x16p.tile([128, KT, TCH], DT16, name="x16")
            nc.sync.dma_start(x0[:], xh_d[:, 0])
            la16 = constp.tile([128, KT, R], DT16)
            nc.sync.dma_start(la16[:], la_d[:])
            sel16 = constp.tile([128, R], DT16)
            nc.sync.dma_start(sel16[:], sel_d[:])
            # t1sb/lb padded to full 128 partitions (zero rows contribute 0)
            # so the fused LoRA+bias matmul is a uniform K=128 FWL-eligible op
            t1sb = constp.tile([128, T], DT16)
            nc.gpsimd.memset(t1sb[:], 0.0)
            nc.sync.dma_start(t1sb[R : R + 1, :], ones_d[:])
            lb16 = constp.tile([128, O_SH], DT16)
            nc.gpsimd.memset(lb16[:], 0.0)
            nc.sync.dma_start(lb16[0 : R + 1, :], lb_d[:])
            s16 = constp.tile([128, O_SH], DT16)
            z16 = constp.tile([128, O_SH], DT16)
            nc.sync.dma_start(s16[:], sb_d[:])
            nc.sync.dma_start(z16[:], zb_d[:])

            def t1_chain(x16, tci):
                # 4 k-tiles concurrently in 32-wide column strips
                t1p = pslp.tile([128, TCH], F32, tag="t1")
                for kk in range(KT // 4):
                    for j in range(4):
                        nc.tensor.matmul(
                            t1p[32 * j : 32 * j + R, :],
                            la16[:, kk * 4 + j, :],
                            x16[:, kk * 4 + j, :],
                            start=(kk == 0),
                            stop=(kk == KT // 4 - 1),
                            tile_position=(0, 32 * j),
                        )
                t1c = t1cp.tile([128, TCH], DT16)
                nc.vector.tensor_copy(t1c[:], t1p[:])
                t1f = pslp.tile([R, TCH], F32, tag="t1")
                nc.tensor.matmul(t1f[:], sel16[:], t1c[:], start=True, stop=True)
                nc.vector.tensor_copy(t1sb[0:R, tci * TCH : (tci + 1) * TCH], t1f[:])

            def load_quarter(q):
                # 4 pipelined sub-DMAs; dequant 8 k-tiles per op in place with
                # stride-0-broadcast scale/zero slices (DVE throughput floor)
                w16 = w16p.tile([128, KT, 512], DT16)
                zq = z16[:, q * 512 : (q + 1) * 512].unsqueeze(1).broadcast_to(
                    (128, 8, 512)
                )
                sq = s16[:, q * 512 : (q + 1) * 512].unsqueeze(1).broadcast_to(
                    (128, 8, 512)
                )
                for kk in range(4):
                    wsl = w16[:, kk * 8 : (kk + 1) * 8, :]
                    nc.scalar.dma_start(wsl, wq_d[:, q, kk * 8 : (kk + 1) * 8])
                    nc.vector.tensor_sub(wsl, wsl, zq)
                    nc.vector.tensor_mul(wsl, wsl, sq)
                return w16

            t1_chain(x0, 0)

            wt = [load_quarter(0), load_quarter(1), None, None]

            for oh in range(2):
                for tci in range(NTCH):
                    if oh == 0 and tci == 0:
                        x16 = x0
                    else:
                        x16 = x16p.tile([128, KT, TCH], DT16)
                        nc.sync.dma_start(x16[:], xh_d[:, tci])
                        if oh == 0:
                            t1_chain(x16, tci)
                    if oh == 0 and tci in (2, 3):
                        wt[tci] = load_quarter(tci)
                    for tt in range(TCH // 128):
                        t0 = tci * TCH + tt * 128
                        for qq in range(2):
                            q = oh * 2 + qq
                            ps = psp.tile([128, 512], F32)
                            for k in range(KT):
                                nc.tensor.matmul(
                                    ps[:],
                                    x16[:, k, tt * 128 : tt * 128 + 128],
                                    wt[q][:, k, :],
                                    start=(k == 0),
                                    stop=False,
                                )
                            nc.tensor.matmul(
                                ps[:],
                                t1sb[:, t0 : t0 + 128],
                                lb16[:, q * 512 : (q + 1) * 512],
                                start=False,
                                stop=True,
                            )
                            ob = obp.tile([128, 512], F32)
                            nc.scalar.copy(ob[:], ps[:])
                            nc.sync.dma_start(out_d[q, t0 : t0 + 128, :], ob[:])

    nc.compile()
    return nc


def kernel(x, W_q, scale, zero, lora_A, lora_B, bias):
    global _nc_cache, LAST_RESULTS
    if _nc_cache is None:
        _nc_cache = _build()
    nc = _nc_cache

    x = np.asarray(x, dtype=np.float32)
    W_q = np.asarray(W_q, dtype=np.int32)
    scale = np.asarray(scale, dtype=np.float32)
    zero = np.asarray(zero, dtype=np.float32)
    lora_A = np.asarray(lora_A, dtype=np.float32)
    lora_B = np.asarray(lora_B, dtype=np.float32)
    bias = np.asarray(bias, dtype=np.float32)

    xh_b = []
    for b in range(B):
        xT = x[b].T[PERM]  # [I, T]
        xh = xT.reshape(KT, 128, NTCH, TCH).transpose(1, 2, 0, 3)
        xh_b.append(np.ascontiguousarray(xh.astype(NP16)))
    la = (SCALING * lora_A)[PERM].reshape(KT, 128, R).transpose(1, 0, 2)
    la = np.ascontiguousarray(la.astype(NP16))
    sel = np.zeros((128, R), dtype=NP16)
    for j in range(4):
        sel[32 * j + np.arange(R), np.arange(R)] = 1

    in_maps = []
    for c in range(NCORES):
        b, og = c // OG, c % OG
        osl = slice(og * O_SH, (og + 1) * O_SH)
        wq = W_q[osl].T[PERM].reshape(KT, 128, NQ, 512).transpose(1, 2, 0, 3)
        sb = np.ascontiguousarray(scale[osl].T)[np.arange(128) % GS]
        zb = np.ascontiguousarray(zero[osl].T)[np.arange(128) % GS]
        lb = np.concatenate([lora_B[:, osl], bias[None, osl]], axis=0)
        in_maps.append(
            {
                "xh": xh_b[b],
                "wq": np.ascontiguousarray(wq.astype(NP16)),
                "sb": np.ascontiguousarray(sb.astype(NP16)),
                "zb": np.ascontiguousarray(zb.astype(NP16)),
                "la": la,
                "lb": np.ascontiguousarray(lb.astype(NP16)),
                "ones": np.ones((1, T), dtype=NP16),
                "sel": sel,
            }
        )

    res = run_bass_kernel_spmd(
        nc,
        in_maps,
        core_ids=list(range(NCORES)),
        trace=TRACE,
        trace_kwargs=TRACE_KWARGS,
    )
    LAST_RESULTS = res

    out = np.empty((B, S, O), dtype=np.float32)
    for c in range(NCORES):
        b, og = c // OG, c % OG
        o_c = res.results[c]["out"]  # [NQ, T, 512]
        for q in range(NQ):
            o0 = og * O_SH + q * 512
            out[b, :, o0 : o0 + 512] = o_c[q]
    return out
